# revision 5
# baseline (speedup 1.0000x reference)
"""CBPMF mean/std kernel for Trainium2 (8 NeuronCores, data-parallel).

reference semantics:
    mean[b] = dot(U[user_idx[b]], V[item_idx[b]])
    std[b]  = sqrt(1 / (alpha * gamma_u[user_idx[b]] * gamma_v[item_idx[b]]))

Strategy: shard the 1M-interaction batch across 8 cores (131072 each);
replicate the embedding tables. On the host, pack each table with its gamma
column into an augmented [rows, 132] f32 table (128 dims + gamma + 3 zero
pads -> 528B rows) so ONE indirect-DMA gather per interaction fetches both
the embedding row and its gamma. On device: load all indices into resident
SBUF tiles once, then per 4096-interaction chunk gather u/v rows,
elementwise multiply, segmented reduce -> mean, alpha*gu*gv -> reciprocal
-> sqrt -> std.

Device-side layout: the core's 131072 interactions are viewed as
[128 partitions, 1024]; chunk c covers columns [c*32, (c+1)*32). So
interaction (p, f) corresponds to flat index p*1024 + f on both the index
inputs and the mean/std outputs.
"""

import numpy as np

NUM_USERS = 100000
NUM_ITEMS = 50000
D = 128
DAUG = 132  # 128 dims + gamma + 3 pads; 528B rows (16B aligned, >=512B desc)
N_CORES = 8
B = 1048576
B_CORE = B // N_CORES  # 131072

F_CORE = B_CORE // 128  # 1024 interactions per partition
ROWS_PER_PART = 32  # gathered rows per partition per chunk
CHUNK = 128 * ROWS_PER_PART  # 4096 interactions per chunk
N_CHUNKS = B_CORE // CHUNK  # 32

# test.py toggles these for profiling; the grading path leaves them alone.
TRACE = False
TRACE_KWARGS = {}
LAST_RESULTS = None

_NC_CACHE = {}


def _build_nc():
    import concourse.bacc as bacc
    import concourse.bass as bass
    import concourse.mybir as mybir
    import concourse.tile as tile

    f32 = mybir.dt.float32
    i32 = mybir.dt.int32
    n = ROWS_PER_PART

    nc = bacc.Bacc()

    uidx = nc.dram_tensor("uidx", [B_CORE], i32, kind="ExternalInput")
    iidx = nc.dram_tensor("iidx", [B_CORE], i32, kind="ExternalInput")
    utab = nc.dram_tensor("utab", [NUM_USERS, DAUG], f32, kind="ExternalInput")
    vtab = nc.dram_tensor("vtab", [NUM_ITEMS, DAUG], f32, kind="ExternalInput")
    alpha = nc.dram_tensor("alpha", [128, 1], f32, kind="ExternalInput")
    mean_o = nc.dram_tensor("mean", [B_CORE], f32, kind="ExternalOutput")
    std_o = nc.dram_tensor("std", [B_CORE], f32, kind="ExternalOutput")

    uidx_pf = uidx[:].rearrange("(p f) -> p f", p=128)
    iidx_pf = iidx[:].rearrange("(p f) -> p f", p=128)
    mean_pf = mean_o[:].rearrange("(p f) -> p f", p=128)
    std_pf = std_o[:].rearrange("(p f) -> p f", p=128)

    with tile.TileContext(nc) as tc:
        with (
            tc.tile_pool(name="const", bufs=1) as const_pool,
            tc.tile_pool(name="rows", bufs=3) as row_pool,
            tc.tile_pool(name="small", bufs=3) as small_pool,
        ):
            alpha_t = const_pool.tile([128, 1], f32)
            nc.sync.dma_start(alpha_t[:], alpha[:])

            uidx_t = const_pool.tile([128, F_CORE], i32)
            iidx_t = const_pool.tile([128, F_CORE], i32)
            nc.sync.dma_start(uidx_t[:], uidx_pf)
            nc.sync.dma_start(iidx_t[:], iidx_pf)

            for c in range(N_CHUNKS):
                cols = slice(c * n, (c + 1) * n)

                ut = row_pool.tile([128, n * DAUG], f32, tag="u")
                vt = row_pool.tile([128, n * DAUG], f32, tag="v")
                # HW indirect DMA: one descriptor per dest partition, one
                # index per partition -> gather 128 rows per call into a
                # [128, DAUG] column slice of the chunk tile.
                for j in range(n):
                    col = c * n + j
                    nc.gpsimd.indirect_dma_start(
                        out=ut[:, j * DAUG : (j + 1) * DAUG],
                        out_offset=None,
                        in_=utab[:],
                        in_offset=bass.IndirectOffsetOnAxis(
                            ap=uidx_t[:, col : col + 1], axis=0
                        ),
                    )
                    nc.gpsimd.indirect_dma_start(
                        out=vt[:, j * DAUG : (j + 1) * DAUG],
                        out_offset=None,
                        in_=vtab[:],
                        in_offset=bass.IndirectOffsetOnAxis(
                            ap=iidx_t[:, col : col + 1], axis=0
                        ),
                    )

                # ut <- ut * vt elementwise; block k of 132 then holds
                # [u*v (128 floats), gu*gv, 0, 0, 0] for interaction (p, k).
                nc.vector.tensor_mul(ut[:], ut[:], vt[:])

                pv = ut[:].rearrange("p (n d) -> p n d", d=DAUG)

                mean_t = small_pool.tile([128, n], f32, tag="mean")
                nc.vector.reduce_sum(
                    out=mean_t[:],
                    in_=pv[:, :, 0:D],
                    axis=mybir.AxisListType.X,
                )

                prec_t = small_pool.tile([128, n], f32, tag="prec")
                nc.vector.tensor_scalar_mul(prec_t[:], pv[:, :, D : D + 1], alpha_t[:])
                inv_t = small_pool.tile([128, n], f32, tag="inv")
                nc.vector.reciprocal(inv_t[:], prec_t[:])
                std_t = small_pool.tile([128, n], f32, tag="std")
                nc.scalar.sqrt(std_t[:], inv_t[:])

                nc.sync.dma_start(mean_pf[:, cols], mean_t[:])
                nc.sync.dma_start(std_pf[:, cols], std_t[:])

    nc.finalize()
    return nc


def kernel(user_idx, item_idx, U, V, alpha, gamma_u, gamma_v):
    global LAST_RESULTS
    from concourse.bass_utils import run_bass_kernel_spmd

    ui = np.ascontiguousarray(np.asarray(user_idx).astype(np.int32))
    ii = np.ascontiguousarray(np.asarray(item_idx).astype(np.int32))
    U = np.asarray(U, dtype=np.float32)
    V = np.asarray(V, dtype=np.float32)
    gu = np.asarray(gamma_u, dtype=np.float32)
    gv = np.asarray(gamma_v, dtype=np.float32)
    a = float(np.asarray(alpha, dtype=np.float32).reshape(-1)[0])

    utab = np.zeros((NUM_USERS, DAUG), dtype=np.float32)
    utab[:, :D] = U
    utab[:, D] = gu
    vtab = np.zeros((NUM_ITEMS, DAUG), dtype=np.float32)
    vtab[:, :D] = V
    vtab[:, D] = gv
    alpha_arr = np.full((128, 1), a, dtype=np.float32)

    if "nc" not in _NC_CACHE:
        _NC_CACHE["nc"] = _build_nc()
    nc = _NC_CACHE["nc"]

    in_maps = [
        {
            "uidx": ui[c * B_CORE : (c + 1) * B_CORE],
            "iidx": ii[c * B_CORE : (c + 1) * B_CORE],
            "utab": utab,
            "vtab": vtab,
            "alpha": alpha_arr,
        }
        for c in range(N_CORES)
    ]

    res = run_bass_kernel_spmd(
        nc,
        in_maps,
        core_ids=list(range(N_CORES)),
        trace=TRACE,
        **TRACE_KWARGS,
    )
    LAST_RESULTS = res

    mean = np.concatenate([r["mean"] for r in res.results])
    std = np.concatenate([r["std"] for r in res.results])
    return (mean, std)
